# revision 1
# baseline (speedup 1.0000x reference)
"""Trainium2 Bass kernel for nn_CERLoss (CER / Levenshtein DP loss).

Strategy (8 NeuronCores, data-parallel over batch):
  - Each core owns 4 batch rows ([4, 256, 32000] fp32 slab, contiguous).
  - Phase A (memory-bound): stream the slab through SBUF in [128, 4000]
    fp32 tiles (16KB row packets). One 3D-view tensor_reduce per tile
    gives 8 window maxes (window=500) at the same cost as a flat reduce.
    Per 128-row block: row max over the 64 window maxes, locate the
    first window attaining it, indirect-refetch just [128, 500], then
    max8 + max_index give the exact argmax. ~1.016 DMA passes, ~1.09
    DVE passes.
  - Mismatch build (interleaved per block pair): M[(b,i), j] =
    (t_i != idx_j) + 512 - 514*w_i, staged to DRAM in per-b DP layout
    [4, 256, 258] (fp16).
  - Phase B: Levenshtein DP in a shifted domain S[i][j] = D[i][j]-j-c_i.
    S_i[j] = min(S_{i-1}[j], S_{i-1}[j-1]+M_i[j], S_i[j-1]) maps onto
    2 DVE instructions per target step (fp16 add + tensor_tensor_scan
    (min, min)). G rows stream back from DRAM in 64-row tiles,
    double-buffered. Values stay integral, |.| <= 2048: fp16 exact.
  - loss_row = S_final[len] + 2*len; host averages the 32 row losses.
"""

import numpy as np

B, S, V = 32, 256, 32000
NCORES = 8
BC = B // NCORES            # batch rows per core = 4
ROWS = BC * S               # (b, s) rows per core = 1024
NBLK = ROWS // 128          # row blocks of 128 partitions = 8
VT = 4000                   # vocab tile width (fp32 -> 16KB DMA packets)
NT = V // VT                # tiles per row block = 8
WIN = 500                   # argmax window
NW = V // WIN               # windows per row = 64
BIG = 512.0
J1 = S + 1                  # 257 DP columns
GW = S + 2                  # 258-wide padded rows in G
GROWS = 64                  # G rows streamed per DP tile

_cache = {}


def _build():
    import sys
    if '/opt/trn_rl_repo' not in sys.path:
        sys.path.insert(0, '/opt/trn_rl_repo')
    import concourse.bass as bass
    import concourse.bacc as bacc
    import concourse.mybir as mybir
    import concourse.tile as tile

    fp32 = mybir.dt.float32
    fp16 = mybir.dt.float16
    i32 = mybir.dt.int32
    u32 = mybir.dt.uint32
    Alu = mybir.AluOpType
    AX = mybir.AxisListType.X

    nc = bacc.Bacc(None, target_bir_lowering=False, debug=False)
    x = nc.dram_tensor("input", [BC, S, V], fp32, kind="ExternalInput")
    tg = nc.dram_tensor("target", [BC, S], fp32, kind="ExternalInput")
    out = nc.dram_tensor("loss_part", [BC, 1], fp32, kind="ExternalOutput")

    idxd = nc.dram_tensor("idx_scratch", [BC, S], fp32, kind="Internal")
    g3 = nc.dram_tensor("g_scratch", [BC, S, GW], fp16, kind="Internal")

    x_rows = x[:, :, :].rearrange("b s v -> (b s) v")              # [1024, 32000]
    x_wins = x[:, :, :].rearrange("b s (w c) -> (b s w) c", c=WIN)  # [65536, 500]
    tg_flat = tg[:, :].rearrange("b (s u) -> (b s) u", u=1)        # [1024, 1]

    with tile.TileContext(nc) as tc:
        with tc.tile_pool(name="persist", bufs=1) as cpool, \
             tc.tile_pool(name="chunks", bufs=6) as chpool, \
             tc.tile_pool(name="gtiles", bufs=2) as gpool, \
             tc.tile_pool(name="work", bufs=2) as wpool:

            # ---- constants ----
            # descending selection weights 64..1 (first window wins ties)
            w64_i = cpool.tile([128, NW], i32, tag="w64_i")
            nc.gpsimd.iota(w64_i[:, :], pattern=[[-1, NW]], base=NW,
                           channel_multiplier=0)
            w64 = cpool.tile([128, NW], fp32, tag="w64")
            nc.vector.tensor_copy(out=w64[:, :], in_=w64_i[:, :])

            iota_j_i = cpool.tile([BC, J1], i32, tag="iota_j_i")
            nc.gpsimd.iota(iota_j_i[:, :], pattern=[[1, J1]], base=0,
                           channel_multiplier=0)
            iota_j = cpool.tile([BC, J1], fp32, tag="iota_j")
            nc.vector.tensor_copy(out=iota_j[:, :], in_=iota_j_i[:, :])

            # DP state buffers, initialized up front
            sa = cpool.tile([BC, GW], fp16, tag="sa")
            sb = cpool.tile([BC, GW], fp16, tag="sb")
            nc.vector.memset(sa[:, :], 0.0)
            nc.vector.memset(sa[:, 0:1], BIG)
            nc.vector.memset(sb[:, 0:1], BIG)

            # ---- Phase A: argmax over vocab (windowed reduce + refetch) ----
            # The idxd write + mismatch DMAs for block k are deferred until
            # after block k+1's chunk loads are enqueued: their data is then
            # long ready, so they never head-of-line-block the stream queue.
            def flush_block(pk, pidxg):
                pbk, phalf = pk // 2, pk % 2
                nc.sync.dma_start(
                    out=idxd[pbk:pbk + 1, 128 * phalf:128 * (phalf + 1)],
                    in_=pidxg[:, :])
                if phalf != 1:
                    return
                for kk in (pk - 1, pk):
                    hh = kk % 2
                    idxb = wpool.tile([128, S], fp32, tag="idxb")
                    nc.sync.dma_start(
                        out=idxb[:, :],
                        in_=idxd[pbk:pbk + 1, :].to_broadcast([128, S]))
                    tt_k = wpool.tile([128, 1], fp32, tag="tt_k")
                    nc.sync.dma_start(
                        out=tt_k[:, :],
                        in_=tg_flat[128 * kk:128 * (kk + 1), :])
                    nw_t = wpool.tile([128, 1], fp32, tag="nw_t")
                    nc.vector.tensor_scalar(out=nw_t[:, :], in0=tt_k[:, :],
                                            scalar1=0.0, scalar2=-514.0,
                                            op0=Alu.not_equal, op1=Alu.mult)
                    mbase = wpool.tile([128, 1], fp32, tag="mbase")
                    nc.vector.tensor_scalar(out=mbase[:, :], in0=nw_t[:, :],
                                            scalar1=BIG, scalar2=None,
                                            op0=Alu.add)
                    mt = wpool.tile([128, S], fp16, tag="mt")
                    nc.vector.tensor_scalar(out=mt[:, :], in0=idxb[:, :],
                                            scalar1=tt_k[:, :1],
                                            scalar2=mbase[:, :1],
                                            op0=Alu.not_equal, op1=Alu.add)
                    nc.sync.dma_start(
                        out=g3[pbk:pbk + 1, 128 * hh:128 * (hh + 1), 1:S + 1],
                        in_=mt[:, :])

            pending = None
            part2 = None
            for k in range(NBLK):
                bk, half = k // 2, k % 2
                mall = wpool.tile([128, NW], fp32, tag="mall")
                for c in range(NT):
                    ch = chpool.tile([128, VT], fp32, tag="ch")
                    nc.sync.dma_start(
                        out=ch[:, :],
                        in_=x_rows[128 * k:128 * (k + 1), VT * c:VT * (c + 1)])
                    ch3 = ch[:, :].rearrange("p (w c) -> p w c", c=WIN)
                    nc.vector.tensor_reduce(
                        out=mall[:, 8 * c:8 * (c + 1)], in_=ch3[:, :, :],
                        axis=AX, op=Alu.max)
                if part2 is not None:
                    pk, pwinf, prefetch = part2
                    m8 = wpool.tile([128, 8], fp32, tag="m8")
                    nc.vector.max(out=m8[:, :], in_=prefetch[:, :])
                    i8 = wpool.tile([128, 8], u32, tag="i8")
                    nc.vector.max_index(out=i8[:, :], in_max=m8[:, :],
                                        in_values=prefetch[:, :])
                    idxf = wpool.tile([128, 1], fp32, tag="idxf")
                    nc.vector.tensor_copy(out=idxf[:, :], in_=i8[:, 0:1])
                    idxg = wpool.tile([128, 1], fp32, tag="idxg")
                    nc.vector.tensor_scalar(out=idxg[:, :], in0=pwinf[:, :],
                                            scalar1=float(WIN),
                                            scalar2=idxf[:, :1],
                                            op0=Alu.mult, op1=Alu.add)
                    part2 = None
                    if pending is not None:
                        flush_block(*pending)
                        pending = None
                    pending = (pk, idxg)
                rmax = wpool.tile([128, 1], fp32, tag="rmax")
                nc.vector.tensor_reduce(out=rmax[:, :], in_=mall[:, :],
                                        axis=AX, op=Alu.max)
                eq = wpool.tile([128, NW], fp32, tag="eq")
                nc.vector.tensor_scalar(out=eq[:, :], in0=mall[:, :],
                                        scalar1=rmax[:, :1], scalar2=None,
                                        op0=Alu.is_equal)
                tsel = wpool.tile([128, NW], fp32, tag="tsel")
                nc.vector.tensor_tensor(out=tsel[:, :], in0=eq[:, :],
                                        in1=w64[:, :], op=Alu.mult)
                wmax = wpool.tile([128, 1], fp32, tag="wmax")
                nc.vector.tensor_reduce(out=wmax[:, :], in_=tsel[:, :],
                                        axis=AX, op=Alu.max)
                winf = wpool.tile([128, 1], fp32, tag="winf")
                nc.vector.tensor_scalar(out=winf[:, :], in0=wmax[:, :],
                                        scalar1=-1.0, scalar2=float(NW),
                                        op0=Alu.mult, op1=Alu.add)
                wini = wpool.tile([128, 1], i32, tag="wini")
                nc.vector.tensor_copy(out=wini[:, :], in_=winf[:, :])
                rowi = wpool.tile([128, 1], i32, tag="rowi")
                nc.gpsimd.iota(rowi[:, :], pattern=[[0, 1]],
                               base=128 * k * NW, channel_multiplier=NW)
                fetch = wpool.tile([128, 1], i32, tag="fetch")
                nc.vector.tensor_tensor(out=fetch[:, :], in0=rowi[:, :],
                                        in1=wini[:, :], op=Alu.add)
                refetch = wpool.tile([128, WIN], fp32, tag="refetch")
                nc.gpsimd.indirect_dma_start(
                    out=refetch[:, :], out_offset=None,
                    in_=x_wins[:, :],
                    in_offset=bass.IndirectOffsetOnAxis(ap=fetch[:, :1], axis=0))
                part2 = (k, winf, refetch)

            # tail: finish block 7's argmax, then flush both pending blocks
            pk, pwinf, prefetch = part2
            m8 = wpool.tile([128, 8], fp32, tag="m8")
            nc.vector.max(out=m8[:, :], in_=prefetch[:, :])
            i8 = wpool.tile([128, 8], u32, tag="i8")
            nc.vector.max_index(out=i8[:, :], in_max=m8[:, :],
                                in_values=prefetch[:, :])
            idxf = wpool.tile([128, 1], fp32, tag="idxf")
            nc.vector.tensor_copy(out=idxf[:, :], in_=i8[:, 0:1])
            idxg = wpool.tile([128, 1], fp32, tag="idxg")
            nc.vector.tensor_scalar(out=idxg[:, :], in0=pwinf[:, :],
                                    scalar1=float(WIN), scalar2=idxf[:, :1],
                                    op0=Alu.mult, op1=Alu.add)
            flush_block(*pending)
            flush_block(pk, idxg)
            pending = None
            part2 = None

            # ---- extraction precompute (only depends on targets) ----
            tg4 = cpool.tile([BC, S], fp32, tag="tg4")
            nc.sync.dma_start(out=tg4[:, :], in_=tg[:, :])
            wrow = cpool.tile([BC, S], fp32, tag="wrow")
            nc.vector.tensor_scalar(out=wrow[:, :], in0=tg4[:, :],
                                    scalar1=0.0, scalar2=None,
                                    op0=Alu.not_equal)
            lenr = cpool.tile([BC, 1], fp32, tag="lenr")
            nc.vector.tensor_reduce(out=lenr[:, :], in_=wrow[:, :],
                                    axis=AX, op=Alu.add)
            len2 = cpool.tile([BC, 1], fp32, tag="len2")
            nc.vector.tensor_scalar(out=len2[:, :], in0=lenr[:, :],
                                    scalar1=2.0, scalar2=None, op0=Alu.mult)
            eqj = cpool.tile([BC, J1], fp32, tag="eqj")
            nc.vector.tensor_scalar(out=eqj[:, :], in0=iota_j[:, :],
                                    scalar1=lenr[:, :1], scalar2=None,
                                    op0=Alu.is_equal)

            # ---- Phase B: the DP (G rows streamed from DRAM) ----
            ttile = cpool.tile([BC, J1], fp16, tag="ttile")
            cur, nxt = sa, sb
            for t in range(S // GROWS):
                gt = gpool.tile([BC, GROWS * GW], fp16, tag="gt")
                gt3 = gt[:, :].rearrange("p (i j) -> p i j", j=GW)
                nc.sync.dma_start(
                    out=gt[:, :],
                    in_=g3[:, GROWS * t:GROWS * (t + 1), :].rearrange(
                        "b i j -> b (i j)"))
                nc.vector.memset(gt3[:, :, 0:1], BIG)
                for r in range(GROWS):
                    nc.vector.tensor_tensor(out=ttile[:, :], in0=cur[:, 0:J1],
                                            in1=gt[:, r * GW:r * GW + J1],
                                            op=Alu.add)
                    nc.vector.tensor_tensor_scan(out=nxt[:, 1:GW],
                                                 data0=cur[:, 1:GW],
                                                 data1=ttile[:, :],
                                                 initial=BIG,
                                                 op0=Alu.min, op1=Alu.min)
                    cur, nxt = nxt, cur

            # ---- extraction: loss = S_final[len] + 2*len ----
            sf = cpool.tile([BC, J1], fp32, tag="sf")
            nc.vector.tensor_copy(out=sf[:, :], in_=cur[:, 1:GW])
            prod = cpool.tile([BC, J1], fp32, tag="prod")
            nc.vector.tensor_tensor(out=prod[:, :], in0=eqj[:, :],
                                    in1=sf[:, :], op=Alu.mult)
            red = cpool.tile([BC, 1], fp32, tag="red")
            nc.vector.tensor_reduce(out=red[:, :], in_=prod[:, :],
                                    axis=AX, op=Alu.add)
            loss = cpool.tile([BC, 1], fp32, tag="loss")
            nc.vector.tensor_scalar(out=loss[:, :], in0=red[:, :],
                                    scalar1=len2[:, :1], scalar2=None,
                                    op0=Alu.add)
            nc.sync.dma_start(out=out[:, :], in_=loss[:, :])

    nc.compile()
    return nc


def kernel(input, target):
    import sys
    if '/opt/trn_rl_repo' not in sys.path:
        sys.path.insert(0, '/opt/trn_rl_repo')
    from concourse.bass_utils import run_bass_kernel_spmd

    if 'nc' not in _cache:
        _cache['nc'] = _build()
    nc = _cache['nc']

    input = np.ascontiguousarray(np.asarray(input, dtype=np.float32))
    target_f = np.asarray(target).astype(np.float32)

    in_maps = []
    for c in range(NCORES):
        in_maps.append({
            "input": input[BC * c:BC * (c + 1)],
            "target": np.ascontiguousarray(target_f[BC * c:BC * (c + 1)]),
        })
    res = run_bass_kernel_spmd(nc, in_maps, core_ids=list(range(NCORES)))
    parts = [res.results[c]["loss_part"][:, 0] for c in range(NCORES)]
    losses = np.concatenate(parts)
    return np.float32(losses.mean())

